# revision 3
# baseline (speedup 1.0000x reference)
"""Trainium2 Bass kernel for nn_GSPolicyNetLSTM (Gumbel-softmax policy net).

Strategy:
  - The sender/receiver LSTM decode is tiny and fully serial -> replicate it
    on every core (identical compute, no communication).
  - The huge output projection W_r [262144, 100] is sharded row-wise across
    the 8 cores ([32768, 100] each, + bias row).  Its DMA streams in the
    background while the LSTM decode runs.
  - Each core returns exp(logits_shard) [128, 256] plus the shard sum; the
    host concatenates and divides by the global sum (softmax denominator).

Key simplifications (exact in forward pass):
  - st = hard + soft - stop_grad(soft) == hard  -> emitted symbols are exact
    one-hots, so W_ih @ sym is a column gather (register-indexed slice).
  - Post-EOS sender state is dead (receiver updates are gated by `valid`),
    so sender h/c/idx never need freezing.
  - At t==29 the reference emits the EOS constant, so the receiver's last
    input is the static EOS column.
"""

import numpy as np

IN_SHAPE = 4096
H_S = 250
H_R = 100
MAX_LEN = 30
VOCAB = 40
OUT_CLS = 512 ** 2
N_CORES = 8
SHARD = OUT_CLS // N_CORES          # 32768
NJ = SHARD // 128                   # 256 matmul column-tiles per core

_CACHE = {}


def _build_program():
    import concourse.bacc as bacc
    import concourse.bass as bass
    import concourse.mybir as mybir
    import concourse.tile as tile

    f32 = mybir.dt.float32
    u32 = mybir.dt.uint32
    AF = mybir.ActivationFunctionType
    ALU = mybir.AluOpType

    nc = bacc.Bacc("TRN2", target_bir_lowering=False, debug=False,
                   num_devices=N_CORES)

    # ---- DRAM I/O ----
    d_xT = nc.dram_tensor("xT", [128, 32], f32, kind="ExternalInput")
    d_ws1t = nc.dram_tensor("ws1t", [4096, 250], f32, kind="ExternalInput")
    d_bs1 = nc.dram_tensor("bs1", [1, 250], f32, kind="ExternalInput")
    d_whh1t = nc.dram_tensor("whh1t", [250, 1024], f32, kind="ExternalInput")
    d_wih1g = nc.dram_tensor("wih1g", [128, 41 * 8], f32, kind="ExternalInput")
    d_wpt = nc.dram_tensor("wpt", [250, 40], f32, kind="ExternalInput")
    d_gz = nc.dram_tensor("gz", [1, MAX_LEN * 40], f32, kind="ExternalInput")
    d_whh2t = nc.dram_tensor("whh2t", [100, 400], f32, kind="ExternalInput")
    d_wih2g = nc.dram_tensor("wih2g", [100, 41 * 4], f32, kind="ExternalInput")
    d_wrt = nc.dram_tensor("wrt", [101, SHARD], f32, kind="ExternalInput")
    d_oexp = nc.dram_tensor("out_exp", [128, NJ], f32, kind="ExternalOutput")
    d_osum = nc.dram_tensor("out_s", [1, 1], f32, kind="ExternalOutput")

    with tile.TileContext(nc) as tc:
        with (
            tc.tile_pool(name="const", bufs=1) as cpool,
            tc.tile_pool(name="state", bufs=1) as spool,
            tc.tile_pool(name="tmp", bufs=3) as tpool,
        ):
            # ---- SBUF residents ----
            xT = cpool.tile([128, 32], f32, tag="xT")
            ws1t = cpool.tile([128, 32, 250], f32, tag="ws1t")
            bs1 = cpool.tile([1, 250], f32, tag="bs1")
            whh1a = cpool.tile([128, 1024], f32, tag="whh1a")
            whh1b = cpool.tile([122, 1024], f32, tag="whh1b")
            wih1g = cpool.tile([128, 41 * 8], f32, tag="wih1g")
            wpta = cpool.tile([128, 40], f32, tag="wpta")
            wptb = cpool.tile([122, 40], f32, tag="wptb")
            gz = cpool.tile([1, MAX_LEN * 40], f32, tag="gz")
            whh2t = cpool.tile([100, 400], f32, tag="whh2t")
            wih2g = cpool.tile([100, 41 * 4], f32, tag="wih2g")
            wrt = cpool.tile([101, SHARD], f32, tag="wrt")
            ones_row = cpool.tile([1, 128], f32, tag="ones_row")
            one1 = cpool.tile([1, 1], f32, tag="one1")
            ones_col = cpool.tile([128, 1], f32, tag="ones_col")

            h_a = spool.tile([128, 1], f32, tag="h_a")
            h_b = spool.tile([122, 1], f32, tag="h_b")
            c_st = spool.tile([128, 2], f32, tag="c_st")
            done = spool.tile([128, 1], f32, tag="done")
            h2 = spool.tile([100, 1], f32, tag="h2")
            c2 = spool.tile([100, 1], f32, tag="c2")
            h2aug = spool.tile([101, 1], f32, tag="h2aug")
            h0row = spool.tile([1, 256], f32, tag="h0row")

            # ---- input DMAs (small first, big W_r last so it streams in
            # the background on the same HWDGE FIFO) ----
            nc.sync.dma_start(xT[:], d_xT[:])
            nc.sync.dma_start(bs1[:], d_bs1[:])
            nc.sync.dma_start(whh1a[:], d_whh1t[0:128, :])
            nc.sync.dma_start(whh1b[:], d_whh1t[128:250, :])
            nc.sync.dma_start(wih1g[:], d_wih1g[:])
            nc.sync.dma_start(wpta[:], d_wpt[0:128, :])
            nc.sync.dma_start(wptb[:], d_wpt[128:250, :])
            nc.sync.dma_start(gz[:], d_gz[:])
            nc.sync.dma_start(whh2t[:], d_whh2t[:])
            nc.sync.dma_start(wih2g[:], d_wih2g[:])
            ws1t_re = d_ws1t.ap().rearrange("(c p) j -> p c j", p=128)
            for cc in range(4):
                nc.sync.dma_start(ws1t[:, 8 * cc:8 * cc + 8, :],
                                  ws1t_re[:, 8 * cc:8 * cc + 8, :])
            nc.sync.dma_start(wrt[:], d_wrt[:])

            # ---- constants / state init ----
            nc.vector.memset(ones_row[:], 1.0)
            nc.vector.memset(one1[:], 1.0)
            nc.vector.memset(ones_col[:], 1.0)
            nc.vector.memset(done[:], 0.0)
            nc.vector.memset(h2[:], 0.0)
            nc.vector.memset(c2[:], 0.0)
            nc.vector.memset(c_st[:], 0.0)
            nc.vector.memset(h2aug[:], 1.0)  # row 100 stays 1.0 (bias)

            # ---- phase 1: h0 = relu(W_s1 @ x + b_s1) ----
            with tc.tile_pool(name="ph0", bufs=2,
                              space=bass.MemorySpace.PSUM) as p0:
                h0ps = p0.tile([1, 256], f32, tag="h0ps")
                nc.tensor.matmul(h0ps[0:1, 0:250], one1[:], bs1[:],
                                 start=True, stop=False)
                for c in range(32):
                    nc.tensor.matmul(h0ps[0:1, 0:250], xT[:, c:c + 1],
                                     ws1t[:, c, :], start=False,
                                     stop=(c == 31))
                nc.scalar.activation(h0row[0:1, 0:250], h0ps[0:1, 0:250],
                                     AF.Relu)
                tra = p0.tile([128, 1], f32, tag="trps")
                nc.tensor.matmul(tra[:], h0row[0:1, 0:128], one1[:],
                                 start=True, stop=True)
                nc.vector.tensor_copy(h_a[:], tra[:])
                trb = p0.tile([128, 1], f32, tag="trps")
                nc.tensor.matmul(trb[0:122, :], h0row[0:1, 128:250], one1[:],
                                 start=True, stop=True)
                nc.vector.tensor_copy(h_b[:], trb[0:122, :])

            # ---- phase 2: decode loop ----
            with (
                tc.tile_pool(name="psA", bufs=2,
                             space=bass.MemorySpace.PSUM) as psA,
                tc.tile_pool(name="psZ", bufs=2,
                             space=bass.MemorySpace.PSUM) as psZ,
                tc.tile_pool(name="psE", bufs=1,
                             space=bass.MemorySpace.PSUM) as psE,
                tc.tile_pool(name="psG2", bufs=2,
                             space=bass.MemorySpace.PSUM) as psG2,
            ):
                rv = None  # ScalarValue of previous step's argmax index
                for t in range(MAX_LEN):
                    # valid_t = 1 - done  (pre-update)
                    vbc = tpool.tile([128, 1], f32, tag="vbc")
                    nc.vector.tensor_scalar(vbc[:], done[:], -1.0, 1.0,
                                            ALU.mult, ALU.add)

                    if t < MAX_LEN - 1:
                        # -- sender LSTM cell --
                        g1 = psA.tile([128, 8], f32, tag="g1")
                        for jj in range(8):
                            nc.tensor.matmul(
                                g1[:, jj:jj + 1],
                                whh1a[:, 128 * jj:128 * (jj + 1)], h_a[:],
                                start=True, stop=False)
                            nc.tensor.matmul(
                                g1[:, jj:jj + 1],
                                whh1b[:, 128 * jj:128 * (jj + 1)], h_b[:],
                                start=False, stop=True)
                        if t == 0:
                            ihsl = wih1g[:, 40 * 8:41 * 8]   # SOS slot
                        else:
                            ihsl = wih1g[:, bass.ts(rv, 8)]
                        gsum = tpool.tile([128, 8], f32, tag="gsum")
                        nc.vector.tensor_add(gsum[:], g1[:], ihsl)
                        act = tpool.tile([128, 8], f32, tag="act")
                        nc.scalar.activation(act[:, 0:6], gsum[:, 0:6],
                                             AF.Sigmoid)
                        nc.scalar.activation(act[:, 6:8], gsum[:, 6:8],
                                             AF.Tanh)
                        fc = tpool.tile([128, 2], f32, tag="fc")
                        nc.vector.tensor_mul(fc[:], act[:, 2:4], c_st[:])
                        ig = tpool.tile([128, 2], f32, tag="ig")
                        nc.vector.tensor_mul(ig[:], act[:, 0:2], act[:, 6:8])
                        nc.vector.tensor_add(c_st[:], fc[:], ig[:])
                        tch = tpool.tile([128, 2], f32, tag="tch")
                        nc.scalar.activation(tch[:], c_st[:], AF.Tanh)
                        nc.vector.tensor_mul(h_a[:], act[:, 4:5],
                                             tch[:, 0:1])
                        nc.vector.tensor_mul(h_b[:], act[0:122, 5:6],
                                             tch[0:122, 1:2])

                        # -- logits + gumbel + argmax --
                        zps = psZ.tile([1, 40], f32, tag="zps")
                        nc.tensor.matmul(zps[:], h_a[:], wpta[:],
                                         start=True, stop=False)
                        nc.tensor.matmul(zps[:], h_b[:], wptb[:],
                                         start=False, stop=True)
                        zsb = tpool.tile([1, 40], f32, tag="zsb")
                        nc.vector.tensor_add(zsb[:], zps[:],
                                             gz[0:1, 40 * t:40 * (t + 1)])
                        mx8 = tpool.tile([1, 8], f32, tag="mx8")
                        nc.vector.max(mx8[:], zsb[:])
                        idx8 = tpool.tile([1, 8], u32, tag="idx8")
                        nc.vector.max_index(idx8[:], mx8[:], zsb[:])
                        reg = nc.alloc_register(mybir.EngineType.DVE,
                                                f"ridx{t}")
                        nc.vector.reg_load(reg, idx8[0:1, 0:1])
                        rv = nc.snap(reg, donate=True, min_val=0,
                                     max_val=VOCAB - 1)

                        # -- done |= (z[eos] == max) broadcast --
                        eos1 = tpool.tile([1, 1], f32, tag="eos1")
                        nc.vector.tensor_scalar(eos1[:], zsb[0:1, 39:40],
                                                mx8[0:1, 0:1], None,
                                                ALU.is_equal)
                        ebc = psE.tile([128, 1], f32, tag="ebc")
                        nc.tensor.matmul(ebc[:], ones_row[:], eos1[:],
                                         start=True, stop=True)
                        nc.vector.tensor_max(done[:], done[:], ebc[:])

                    # -- receiver LSTM cell (input: msg_t one-hot) --
                    g2 = psG2.tile([100, 4], f32, tag="g2")
                    for g in range(4):
                        nc.tensor.matmul(g2[:, g:g + 1],
                                         whh2t[:, 100 * g:100 * (g + 1)],
                                         h2[:], start=True, stop=True)
                    if t == MAX_LEN - 1:
                        ihsl2 = wih2g[:, 39 * 4:40 * 4]  # forced EOS
                    else:
                        ihsl2 = wih2g[:, bass.ts(rv, 4)]
                    gsum2 = tpool.tile([100, 4], f32, tag="gsum2")
                    nc.vector.tensor_add(gsum2[:], g2[:], ihsl2)
                    act2 = tpool.tile([100, 4], f32, tag="act2")
                    nc.scalar.activation(act2[:, 0:3], gsum2[:, 0:3],
                                         AF.Sigmoid)
                    nc.scalar.activation(act2[:, 3:4], gsum2[:, 3:4],
                                         AF.Tanh)
                    fc2 = tpool.tile([100, 1], f32, tag="fc2")
                    nc.vector.tensor_mul(fc2[:], act2[:, 1:2], c2[:])
                    ig2 = tpool.tile([100, 1], f32, tag="ig2")
                    nc.vector.tensor_mul(ig2[:], act2[:, 0:1], act2[:, 3:4])
                    c2n = tpool.tile([100, 1], f32, tag="c2n")
                    nc.vector.tensor_add(c2n[:], fc2[:], ig2[:])
                    tc2 = tpool.tile([100, 1], f32, tag="tc2")
                    nc.scalar.activation(tc2[:], c2n[:], AF.Tanh)
                    h2n = tpool.tile([100, 1], f32, tag="h2n")
                    nc.vector.tensor_mul(h2n[:], act2[:, 2:3], tc2[:])
                    vmask = vbc[0:100, :].bitcast(mybir.dt.int32)
                    nc.vector.copy_predicated(c2[:], vmask, c2n[:])
                    nc.vector.copy_predicated(h2[:], vmask, h2n[:])

            # ---- phase 3: logits shard = W_r @ hR + b_r; exp + sum ----
            nc.vector.tensor_copy(h2aug[0:100, :], h2[:])
            with tc.tile_pool(name="psW", bufs=1,
                              space=bass.MemorySpace.PSUM) as psW:
                lg = psW.tile([128, NJ], f32, tag="lg")
                for j in range(NJ):
                    nc.tensor.matmul(lg[:, j:j + 1],
                                     wrt[:, 128 * j:128 * (j + 1)],
                                     h2aug[:], start=True, stop=True)
                expt = spool.tile([128, NJ], f32, tag="expt")
                rsum = spool.tile([128, 1], f32, tag="rsum")
                nc.scalar.activation(expt[:], lg[:], AF.Exp,
                                     accum_out=rsum[:])
                sps = psW.tile([1, 1], f32, tag="sps")
                nc.tensor.matmul(sps[:], rsum[:], ones_col[:],
                                 start=True, stop=True)
                ssb = spool.tile([1, 1], f32, tag="ssb")
                nc.vector.tensor_copy(ssb[:], sps[:])
                nc.sync.dma_start(d_oexp[:], expt[:])
                nc.sync.dma_start(d_osum[:], ssb[:])

    nc.compile()
    return nc


def _prep_inputs(inputs):
    """Host-side tensor layout prep.  Returns per-core input maps."""
    f = lambda k: np.asarray(inputs[k], dtype=np.float32)
    x = f("x"); gumbel = f("gumbel_noise")
    W_s1 = f("W_s1"); b_s1 = f("b_s1")
    W_ih1 = f("W_ih1"); W_hh1 = f("W_hh1")
    b1 = f("b_ih1") + f("b_hh1")
    W_p = f("W_p"); b_p = f("b_p")
    W_ih2 = f("W_ih2"); W_hh2 = f("W_hh2")
    b2 = f("b_ih2") + f("b_hh2")
    W_r = f("W_r"); b_r = f("b_r")

    GORD = (0, 1, 3, 2)  # torch (i,f,g,o) -> ours (i,f,o,g)

    def perm1(v):  # [1000,...] -> [1024,...] gate-reordered+padded
        out = np.zeros((1024,) + v.shape[1:], np.float32)
        for k, G in enumerate(GORD):
            out[256 * k:256 * k + 250] = v[250 * G:250 * G + 250]
        return out

    def perm2(v):  # [400,...] -> [400,...] gate-reordered
        return np.concatenate([v[100 * G:100 * G + 100] for G in GORD], 0)

    whh1t = np.ascontiguousarray(perm1(W_hh1).T)              # [250,1024]
    wih1_cols = np.concatenate([W_ih1 + b1[:, None],
                                b1[:, None]], axis=1)          # [1000,41]
    wih1g = (perm1(wih1_cols).reshape(8, 128, 41)
             .transpose(1, 2, 0).reshape(128, 41 * 8))
    wih1g = np.ascontiguousarray(wih1g)
    wpt = np.ascontiguousarray(W_p.T)                          # [250,40]
    gz = (gumbel + b_p[None, :]).reshape(1, MAX_LEN * 40)
    whh2t = np.ascontiguousarray(perm2(W_hh2).T)               # [100,400]
    wih2_cols = np.concatenate([W_ih2 + b2[:, None],
                                b2[:, None]], axis=1)          # [400,41]
    wih2g = (perm2(wih2_cols).reshape(4, 100, 41)
             .transpose(1, 2, 0).reshape(100, 41 * 4))
    wih2g = np.ascontiguousarray(wih2g)
    xT = np.ascontiguousarray(x.reshape(32, 128).T)            # [128,32]
    ws1t = np.ascontiguousarray(W_s1.T)                        # [4096,250]
    bs1 = b_s1.reshape(1, 250)
    wrt_full = np.concatenate([W_r.T, b_r[None, :]], 0)        # [101,262144]

    shared = dict(xT=xT, ws1t=ws1t, bs1=np.ascontiguousarray(bs1),
                  whh1t=whh1t, wih1g=wih1g,
                  wpt=wpt, gz=np.ascontiguousarray(gz),
                  whh2t=whh2t, wih2g=wih2g)
    maps = []
    for c in range(N_CORES):
        m = dict(shared)
        m["wrt"] = np.ascontiguousarray(
            wrt_full[:, SHARD * c:SHARD * (c + 1)])
        maps.append(m)
    return maps


def run(inputs, trace=False):
    from concourse.bass_utils import run_bass_kernel_spmd
    if "nc" not in _CACHE:
        _CACHE["nc"] = _build_program()
    nc = _CACHE["nc"]
    maps = _prep_inputs(inputs)
    res = run_bass_kernel_spmd(nc, maps, list(range(N_CORES)), trace=trace)
    parts, total = [], 0.0
    for c in range(N_CORES):
        e = np.asarray(res.results[c]["out_exp"])      # [128, 256]
        parts.append(e.T.reshape(-1))                  # class = 128*j + p
        total += float(np.asarray(res.results[c]["out_s"]).reshape(-1)[0])
    full = np.concatenate(parts).astype(np.float64)
    out = (full / total).astype(np.float32)
    return out, res


def kernel(**inputs):
    out, _ = run(inputs, trace=False)
    return out
